# revision 1
# baseline (speedup 1.0000x reference)
"""Grouped-query attention (B=2, T=2048, d_model=2048, 32 Q heads / 8 KV heads)
sharded over 8 NeuronCores: batch x head-block tensor parallel.

Core c handles batch b = c//4 and head-block hb = c%4 (8 q heads = 2 kv groups).
Everything on-device is feature-major (transposed); the host feeds pre-transposed
inputs and sums/transposes the per-core partial outputs.
"""

import numpy as np

D_MODEL = 2048
T = 2048
B = 2
DK = 64
NREP = 4

MASK_VAL = -400.0  # pre-scale additive mask; exp(0.125 * -400) == 0 effectively

_CACHE: dict = {}


# --------------------------------------------------------------------------
# device kernel
# --------------------------------------------------------------------------
def _build_nc(phases="ABC", reps=1):
    import os
    phases = os.environ.get("K_PHASES", phases)
    reps = int(os.environ.get("K_REPS", reps))
    import concourse.bass as bass
    import concourse.mybir as mybir
    import concourse.tile as tile
    from concourse import bacc
    from concourse.masks import make_identity

    F32 = mybir.dt.float32
    F32R = mybir.dt.float32r
    EXP = mybir.ActivationFunctionType.Exp

    nc = bacc.Bacc("TRN2", target_bir_lowering=False, debug=False)

    xT = nc.dram_tensor("xT", [2048, 2048], F32R, kind="ExternalInput").ap()
    WqT = nc.dram_tensor("WqT", [2048, 512], F32R, kind="ExternalInput").ap()
    WkT = nc.dram_tensor("WkT", [2048, 128], F32R, kind="ExternalInput").ap()
    WvT = nc.dram_tensor("WvT", [2048, 128], F32R, kind="ExternalInput").ap()
    WoT = nc.dram_tensor("WoT", [512, 2048], F32R, kind="ExternalInput").ap()
    MSK = nc.dram_tensor("MSK", [128, 512], F32, kind="ExternalInput").ap()
    YT = nc.dram_tensor("YT", [2048, 2048], F32, kind="ExternalOutput").ap()

    with tile.TileContext(nc) as tc:
        with tc.tile_pool(name="consts", bufs=1) as consts, \
             tc.tile_pool(name="persist", bufs=1) as persist, \
             tc.tile_pool(name="xp", bufs=3) as xp, \
             tc.tile_pool(name="pp", bufs=3) as pp, \
             tc.tile_pool(name="wop", bufs=8) as wop, \
             tc.tile_pool(name="rcp", bufs=4) as rcp, \
             tc.tile_pool(name="qsc", bufs=2) as qsc, \
             tc.tile_pool(name="ytp", bufs=4) as ytp, \
             tc.tile_pool(name="ps", bufs=2, space="PSUM") as ps:

            # ---------------- constants / persistent sbuf ----------------
            wk_sb = consts.tile([128, 2048], F32R)   # col = ck*128 + kvd
            wv_sb = consts.tile([128, 2048], F32R)
            msk_sb = consts.tile([128, 512], F32)
            nc.sync.dma_start(out=msk_sb, in_=MSK)
            # strided loads: chunk ck of WqT (rows ck*128..) -> cols ck*512..
            nc.sync.dma_start(
                out=bass.AP(tensor=wk_sb.tensor, offset=wk_sb.offset,
                            ap=[wk_sb.ap[0], [128, 16], [1, 128]]),
                in_=bass.AP(tensor=WkT.tensor, offset=0,
                            ap=[[128, 128], [128 * 128, 16], [1, 128]]),
            )
            nc.sync.dma_start(
                out=bass.AP(tensor=wv_sb.tensor, offset=wv_sb.offset,
                            ap=[wv_sb.ap[0], [128, 16], [1, 128]]),
                in_=bass.AP(tensor=WvT.tensor, offset=0,
                            ap=[[128, 128], [128 * 128, 16], [1, 128]]),
            )

            idl_f32 = consts.tile([128, 128], F32)
            make_identity(nc, idl_f32)
            idl = consts.tile([128, 128], F32R)
            nc.vector.tensor_copy(idl, idl_f32)
            ones_f32 = consts.tile([128, 1], F32)
            nc.vector.memset(ones_f32, 1.0)
            ones_r = consts.tile([128, 1], F32R)
            nc.vector.tensor_copy(ones_r, ones_f32)

            qt_sb = persist.tile([128, 8192], F32R)   # col = qc*2048 + tok
            kt_sb = persist.tile([128, 2048], F32R)   # [kvd, tok]
            vt_sb = persist.tile([128, 2048], F32R)   # [kvd, tok] (temp for transpose)
            va_all = persist.tile([128, 2080], F32R)  # 32 x [128 tok, 65] V_aug tiles
            ktg1_sb = persist.tile([64, 2048], F32R)  # g1 KT rows shifted to base 0
            otn_sb = persist.tile([128, 8192], F32R)  # col = oc*2048 + tok

            # ---------------- phases (repeated for timing calibration) ----
            for _rep in range(reps):
              # ---------------- phase A: projections ----------------
              for tb in range(4 if "A" in phases else 0):
                  qt_ps = [ps.tile([128, 1024], F32, tag="big", bufs=2, name=f"qtps{tb}_{i}")
                           for i in range(2)]
                  kt_ps = ps.tile([128, 512], F32, tag="sm", bufs=4, name=f"ktps{tb}")
                  vt_ps = ps.tile([128, 512], F32, tag="sm", bufs=4, name=f"vtps{tb}")
                  for ck in range(16):
                      xt = xp.tile([128, 512], F32R, tag="xt", name=f"xt{tb}_{ck}")
                      nc.sync.dma_start(
                          out=xt, in_=xT[128 * ck:128 * (ck + 1), 512 * tb:512 * (tb + 1)])
                      wq_t = xp.tile([128, 512], F32R, tag="wq", bufs=4, name=f"wq{tb}_{ck}")
                      nc.sync.dma_start(out=wq_t, in_=WqT[128 * ck:128 * (ck + 1), :])
                      first, last = ck == 0, ck == 15
                      for qp in range(2):
                          for u in range(2):
                              qc = 2 * qp + u
                              nc.tensor.matmul(
                                  qt_ps[qp][:, u * 512:(u + 1) * 512],
                                  wq_t[:, qc * 128:(qc + 1) * 128],
                                  xt, start=first, stop=last)
                      nc.tensor.matmul(kt_ps, wk_sb[:, ck * 128:(ck + 1) * 128], xt,
                                       start=first, stop=last)
                      nc.tensor.matmul(vt_ps, wv_sb[:, ck * 128:(ck + 1) * 128], xt,
                                       start=first, stop=last)
                  for qp in range(2):
                      dest = bass.AP(tensor=qt_sb.tensor,
                                     offset=qt_sb.offset + (2 * qp) * 2048 + tb * 512,
                                     ap=[qt_sb.ap[0], [2048, 2], [1, 512]])
                      nc.vector.tensor_copy(dest, qt_ps[qp])
                  nc.vector.tensor_copy(kt_sb[:, tb * 512:(tb + 1) * 512], kt_ps)
                  nc.vector.tensor_copy(vt_sb[:, tb * 512:(tb + 1) * 512], vt_ps)

              # stage KT group-1 rows down to partitions 0-63
              nc.gpsimd.dma_start(out=ktg1_sb, in_=kt_sb[64:128, :])

              # V_aug build: transpose VT 128-blocks, append ones column
              for kt in range(16 if "A" in phases else 0):
                  vtp = ps.tile([128, 128], F32R, tag="sm", bufs=4, name=f"vtp{kt}")
                  nc.tensor.transpose(vtp, vt_sb[:, kt * 128:(kt + 1) * 128], idl)
                  for g in range(2):
                      base = (g * 16 + kt) * 65
                      nc.vector.tensor_copy(va_all[:, base:base + 64],
                                            vtp[:, g * 64:(g + 1) * 64])
                      nc.vector.tensor_copy(va_all[:, base + 64:base + 65], ones_r)

              # ---------------- phase B: attention ----------------
              for j in range(8 if "B" in phases else 0):          # query blocks of 256
                  ot = [[ps.tile([65, 512], F32, tag="sm", bufs=4, name=f"ot{j}_{g}_{s}")
                         for s in range(2)] for g in range(2)]
                  nkt = 2 * j + 2
                  # stage g1 query slices (partitions 64-127 -> 0-63) for this j
                  qsl = qsc.tile([64, 1024], F32R, tag="qsl", name=f"qsl{j}")
                  for r in range(4):
                      nc.gpsimd.dma_start(
                          out=qsl[:, r * 256:(r + 1) * 256],
                          in_=qt_sb[64:128, r * 2048 + j * 256: r * 2048 + (j + 1) * 256])
                  for kt in range(nkt):
                      p_kt = pp.tile([128, 2048], F32R, tag="p", name=f"p{j}_{kt}")
                      for g in range(2):
                          st = ps.tile([128, 1024], F32, tag="big", bufs=2, name=f"st{j}_{kt}_{g}")
                          for r in range(4):
                              if g == 0:
                                  rhs = qt_sb[0:64, r * 2048 + j * 256: r * 2048 + (j + 1) * 256]
                                  lhsT = kt_sb[0:64, kt * 128:(kt + 1) * 128]
                              else:
                                  rhs = qsl[:, r * 256:(r + 1) * 256]
                                  lhsT = ktg1_sb[:, kt * 128:(kt + 1) * 128]
                              nc.tensor.matmul(st[:, r * 256:(r + 1) * 256], lhsT, rhs,
                                               start=True, stop=True)
                          if kt >= 2 * j:
                              moff = (kt - 2 * j) * 256
                              mask_b = bass.AP(tensor=msk_sb.tensor,
                                               offset=msk_sb.offset + moff,
                                               ap=[msk_sb.ap[0], [0, 4], [1, 256]])
                              nc.vector.tensor_add(st, st, mask_b)
                          # exp -> P; P col = g*1024 + s*512 + r*128 + qi
                          dest = bass.AP(tensor=p_kt.tensor,
                                         offset=p_kt.offset + g * 1024,
                                         ap=[p_kt.ap[0], [128, 4], [512, 2], [1, 128]])
                          nc.scalar.activation(dest, st, EXP, scale=0.125)
                      for g in range(2):
                          for s in range(2):
                              nc.tensor.matmul(
                                  ot[g][s],
                                  va_all[:, (g * 16 + kt) * 65:(g * 16 + kt) * 65 + 65],
                                  p_kt[:, g * 1024 + s * 512: g * 1024 + (s + 1) * 512],
                                  start=(kt == 0), stop=(kt == nkt - 1))
                  # normalize + evacuate
                  for g in range(2):
                      for s in range(2):
                          rec1 = rcp.tile([1, 512], F32, tag="rec1", name=f"r1_{j}{g}{s}")
                          nc.vector.reciprocal(rec1, ot[g][s][64:65, :])
                          rec_rep = rcp.tile([64, 512], F32, tag="recr", name=f"rr_{j}{g}{s}")
                          nc.gpsimd.partition_broadcast(rec_rep, rec1)
                          for r in range(4):
                              oc = 2 * g + r // 2
                              prow = (r % 2) * 64
                              nc.vector.tensor_mul(
                                  otn_sb[prow:prow + 64,
                                         oc * 2048 + j * 256 + s * 128:
                                         oc * 2048 + j * 256 + (s + 1) * 128],
                                  ot[g][s][0:64, r * 128:(r + 1) * 128],
                                  rec_rep[0:64, r * 128:(r + 1) * 128])

              # ---------------- phase C: output projection ----------------
              for oc in range(16 if "C" in phases else 0):
                  wos = []
                  for odc in range(4):
                      wo_t = wop.tile([128, 128], F32R, tag="wo", name=f"wo{oc}_{odc}")
                      nc.sync.dma_start(
                          out=wo_t,
                          in_=WoT[odc * 128:(odc + 1) * 128, oc * 128:(oc + 1) * 128])
                      wos.append(wo_t)
                  for tb in range(4):
                      yt = ps.tile([128, 512], F32, tag="big", bufs=2, name=f"yt{oc}_{tb}")
                      for odc in range(4):
                          nc.tensor.matmul(yt, wos[odc],
                                           otn_sb[:, odc * 2048 + tb * 512:
                                                  odc * 2048 + (tb + 1) * 512],
                                           start=(odc == 0), stop=(odc == 3))
                      yt_sb = ytp.tile([128, 512], F32, tag="ytsb", name=f"ytsb{oc}_{tb}")
                      nc.vector.tensor_copy(yt_sb, yt)
                      nc.sync.dma_start(
                          out=YT[oc * 128:(oc + 1) * 128, tb * 512:(tb + 1) * 512],
                          in_=yt_sb)

    nc.compile()
    return nc


def _get_nc():
    if "nc" not in _CACHE:
        _CACHE["nc"] = _build_nc()
    return _CACHE["nc"]


# --------------------------------------------------------------------------
# host wrapper
# --------------------------------------------------------------------------
def _make_mask() -> np.ndarray:
    ki = np.arange(128)[:, None]
    qi = np.arange(256)[None, :]
    m0 = np.where(ki <= qi, 0.0, MASK_VAL).astype(np.float32)          # kt == 2j
    m1 = np.where(128 + ki <= qi, 0.0, MASK_VAL).astype(np.float32)    # kt == 2j+1
    return np.concatenate([m0, m1], axis=1)  # [128, 512] = (ktpar, qi)


def _core_inputs(x, Wq, Wk, Wv, Wo, c, mask):
    b, hb = c // 4, c % 4
    xT_c = np.ascontiguousarray(x[b].T)
    # interleave q heads: chunk qc = [g0 rep qc (64) | g1 rep qc (64)]
    g0, g1 = 2 * hb, 2 * hb + 1
    cols = []
    for qc in range(NREP):
        cols.append(Wq[g0 * 256 + qc * 64: g0 * 256 + (qc + 1) * 64])
        cols.append(Wq[g1 * 256 + qc * 64: g1 * 256 + (qc + 1) * 64])
    WqT_c = np.ascontiguousarray(np.concatenate(cols, axis=0).T)
    WkT_c = np.ascontiguousarray(Wk[128 * hb:128 * (hb + 1)].T)
    WvT_c = np.ascontiguousarray(Wv[128 * hb:128 * (hb + 1)].T)
    WoT_c = np.ascontiguousarray(Wo[:, 512 * hb:512 * (hb + 1)].T)
    return {"xT": xT_c, "WqT": WqT_c, "WkT": WkT_c, "WvT": WvT_c,
            "WoT": WoT_c, "MSK": mask}


def kernel(x, Wq, Wk, Wv, Wo, _trace=False, _trace_kwargs=None):
    from concourse import bass_utils

    x = np.asarray(x, dtype=np.float32)
    Wq = np.asarray(Wq, dtype=np.float32)
    Wk = np.asarray(Wk, dtype=np.float32)
    Wv = np.asarray(Wv, dtype=np.float32)
    Wo = np.asarray(Wo, dtype=np.float32)

    nc = _get_nc()
    mask = _make_mask()
    in_maps = [_core_inputs(x, Wq, Wk, Wv, Wo, c, mask) for c in range(8)]

    res = None
    last_exc = None
    for _attempt in range(3):
        try:
            res = bass_utils.run_bass_kernel_spmd(
                nc, in_maps, core_ids=list(range(8)),
                trace=_trace, **(_trace_kwargs or {}))
            break
        except Exception as e:  # transient device wedges happen; retry
            last_exc = e
    if res is None:
        raise last_exc

    Y = np.zeros((B, T, D_MODEL), dtype=np.float32)
    for c in range(8):
        Y[c // 4] += res.results[c]["YT"].T
    if _trace:
        _CACHE["last_result"] = res
    return Y

